# revision 23
# baseline (speedup 1.0000x reference)
"""KroneckerLSTM trn2 kernel.

Computes, for 8 gate-klins (L @ t @ R + b, t in {x,h}):
    i = sigmoid(klin_ii(x) + klin_hi(h)); f = sigmoid(...); g = tanh(...); o = sigmoid(...)
    c_new = f*c + i*g ; h_new = o*tanh(c_new)
Returns (h_new, c_new), each [1024,1024] f32.

Sharding: output rows split across 8 cores (128 rows each) -> zero collectives.
Per core, for each gate g:  B_g[rows,:] = (L_g[rows,:] @ t) @ R_g
  mm1 computes A^T directly (lhsT = t tiles (natural), rhs = host-pretransposed
  L^T column-slices, stacked 4 gates wide so N=512), so mm1's PSUM output is the
  lhsT for mm2 (rhs = R_g in natural layout).  The x-klin and h-klin of each
  gate pair accumulate into the same PSUM bank; bias is added in-place in PSUM.

All DMA'd tensors are bf16 (host-side cast): per-core HBM traffic is ~24MB.
DMA-instruction issue is descriptor-rate-limited (~5ns per partition line), so
all big tensors are host-packed into [128, K*freedim] layouts whose partition
lines are 8KB-contiguous in DRAM; each DMA instruction then moves 0.5-2MB with
only 128 descriptor lines.  Input loads stream on the SP HWDGE ring in
consumption order; output stores use the Activation ring so they never block
input streaming.
"""

import sys

import numpy as np

if "/opt/trn_rl_repo" not in sys.path:
    sys.path.insert(0, "/opt/trn_rl_repo")

N = 1024
M = 1024
P = 128
NC = 8
KT = N // P  # 8 k-tiles of 128
# gate pairs in order i, f, g, o: (x-gate, h-gate, activation)
PAIRS = [("ii", "hi", "Sigmoid"), ("if", "hf", "Sigmoid"),
         ("ig", "hg", "Tanh"), ("io", "ho", "Sigmoid")]

_cache = {}


def _build_program():
    import concourse.bass as bass
    import concourse.mybir as mybir
    import concourse.tile as tile
    from concourse import bacc
    from concourse.bass import ts

    FP = mybir.dt.float32
    BF = mybir.dt.bfloat16
    AF = mybir.ActivationFunctionType

    nc = bacc.Bacc("TRN2", target_bir_lowering=False, debug=False,
                   enable_asserts=False, num_devices=NC)

    # packed layouts: [128, K*freedim], partition p / chunk k = source row k*128+p
    xp_d = nc.dram_tensor("xp", [P, KT * M], BF, kind="ExternalInput").ap()
    hp_d = nc.dram_tensor("hp", [P, KT * M], BF, kind="ExternalInput").ap()
    ltx_d = nc.dram_tensor("ltxp", [P, KT * 512], BF, kind="ExternalInput").ap()
    lth_d = nc.dram_tensor("lthp", [P, KT * 512], BF, kind="ExternalInput").ap()
    rx_d = [nc.dram_tensor(f"rxp{p}", [P, KT * M], BF, kind="ExternalInput").ap()
            for p in range(4)]
    rh_d = [nc.dram_tensor(f"rhp{p}", [P, KT * M], BF, kind="ExternalInput").ap()
            for p in range(4)]
    bs_d = nc.dram_tensor("bsp", [P, 4 * M], BF, kind="ExternalInput").ap()
    c_d = nc.dram_tensor("cprev", [P, M], BF, kind="ExternalInput").ap()
    hn_d = nc.dram_tensor("h_new", [P, M], BF, kind="ExternalOutput").ap()
    cn_d = nc.dram_tensor("c_new", [P, M], BF, kind="ExternalOutput").ap()

    with tile.TileContext(nc) as tc:
        from contextlib import ExitStack
        with ExitStack() as ctx:
            big = ctx.enter_context(tc.tile_pool(name="big", bufs=1))
            atp = ctx.enter_context(tc.tile_pool(name="at", bufs=1))
            rp = ctx.enter_context(tc.tile_pool(name="rstream", bufs=8))
            psp = ctx.enter_context(tc.tile_pool(name="ps", bufs=8, space="PSUM"))
            gp = ctx.enter_context(tc.tile_pool(name="gates", bufs=1))
            ew = ctx.enter_context(tc.tile_pool(name="ew", bufs=1))
            wp = ctx.enter_context(tc.tile_pool(name="warm", bufs=1))

            # small PE warm-up burst overlapping the DMA prologue
            wa = wp.tile([P, P], BF, tag="wa")
            wb = wp.tile([P, 512], BF, tag="wb")
            nc.vector.memset(wa[:], 0.0)
            nc.vector.memset(wb[:], 0.0)
            wps = psp.tile([P, 512], FP, tag="bank", name="warm_ps")
            for w in range(8):
                nc.tensor.matmul(wps[:], wa[:], wb[:], start=True, stop=True,
                                 skip_group_check=True)

            # ---- all input loads, issued upfront in consumption order (SP
            # ring, FIFO).  One DMA per tile (multi-writer tiles add
            # per-reader semaphore overhead); early tiles are small so mm1
            # starts as soon as possible, later ones big to save issue slots.
            xsplit = [(0, 1), (1, 1), (2, 2), (4, 4)]  # (kc0, n_ktiles)
            ltsplit = [(0, 1), (1, 1), (2, 2), (4, 4)]
            xcs, ltxs = [], []
            for i, ((k0, nk), (l0, nl)) in enumerate(zip(xsplit, ltsplit)):
                lt_t = big.tile([P, nl * 512], BF, tag=f"ltx{i}", name=f"ltx{i}")
                ltxs.append((l0, lt_t))
                nc.sync.dma_start(lt_t[:], ltx_d[:, l0 * 512:(l0 + nl) * 512])
                t = big.tile([P, nk * M], BF, tag=f"xc{i}", name=f"xc{i}")
                xcs.append((k0, t))
                nc.sync.dma_start(t[:], xp_d[:, k0 * M:(k0 + nk) * M])
            ltht = big.tile([P, KT * 512], BF, tag="lth")
            nc.sync.dma_start(ltht[:], lth_d[:])
            hcs = []
            for i in range(4):
                t = big.tile([P, 2 * M], BF, tag=f"hc{i}", name=f"hc{i}")
                hcs.append(t)
                nc.sync.dma_start(t[:], hp_d[:, 2 * i * M:2 * (i + 1) * M])
            bst = big.tile([P, 4 * M], BF, tag="bs")
            nc.sync.dma_start(bst[:], bs_d[:])
            cs = ew.tile([P, M], BF, tag="cs")
            nc.sync.dma_start(cs[:], c_d[:])

            def x_ap(kc, j):  # lhsT slice for x k-tile kc, output column j
                for k0, t in reversed(xcs):
                    if kc >= k0:
                        return t[:, (kc - k0) * M + j * P:
                                 (kc - k0) * M + (j + 1) * P]

            def ltx_ap(kc):
                for l0, t in reversed(ltxs):
                    if kc >= l0:
                        return t[:, ts(kc - l0, 512)]

            # mm1-x: at_x[j][mloc, 4*128] = sum_k x[k, j*128+mloc] * LTx[k, col]
            ats = {"x": [None] * KT, "h": [None] * KT}
            pts = [psp.tile([P, 4 * P], FP, tag="bank", name=f"pt_x_{j}")
                   for j in range(KT)]
            for kc in range(KT):
                lts = ltx_ap(kc)
                for j in range(KT):
                    nc.tensor.matmul(pts[j][:], x_ap(kc, j), lts,
                                     start=(kc == 0), stop=(kc == KT - 1))
            for j in range(KT):
                at = atp.tile([P, 4 * P], BF, tag=f"atx{j}", name=f"atx{j}")
                nc.vector.tensor_copy(at[:], pts[j][:])
                ats["x"][j] = at

            # SBUF fp32 partials for the mm2 x-passes (bias folded in)
            pxp = ctx.enter_context(tc.tile_pool(name="px", bufs=1))
            pxs = [pxp.tile([P, M], BF, tag=f"px{p}", name=f"px{p}")
                   for p in range(4)]
            gates = [gp.tile([P, M], BF, tag=f"g{p}", name=f"g{p}")
                     for p in range(4)]

            def mm1h_col(j):
                # one mm1-h output column j (8 MMs, depends only on h/lth):
                # padding work in front of each R-chunk wait so the PE never
                # idles long enough to trip the HAM power throttle
                pw = psp.tile([P, 4 * P], FP, tag="bank", name=f"pt_h_{j}")
                for kc in range(KT):
                    nc.tensor.matmul(pw[:], hcs[kc // 2][:, (kc % 2) * M + j * P:
                                                         (kc % 2) * M + (j + 1) * P],
                                     ltht[:, ts(kc, 512)],
                                     start=(kc == 0), stop=(kc == KT - 1))
                at = atp.tile([P, 4 * P], BF, tag=f"ath{j}", name=f"ath{j}")
                nc.vector.tensor_copy(at[:], pw[:])
                ats["h"][j] = at

            px_banks = {}

            def pair_x_chunk(p, b):
                if b == 0:
                    px_banks[p] = (
                        psp.tile([P, 512], FP, tag="bank", name=f"p{p}xb0"),
                        psp.tile([P, 512], FP, tag="bank", name=f"p{p}xb1"))
                pt0, pt1 = px_banks[p]
                rt = rp.tile([P, 4 * M], BF, tag="r")
                nc.sync.dma_start(rt[:], rx_d[p][:, ts(b, 4 * M)])
                for jj in range(4):
                    j = 4 * b + jj
                    lhsT = ats["x"][j][:, ts(p, P)]
                    nc.tensor.matmul(pt0[:], lhsT,
                                     rt[:, jj * M: jj * M + 512],
                                     start=(j == 0), stop=(j == KT - 1))
                    nc.tensor.matmul(pt1[:], lhsT,
                                     rt[:, jj * M + 512: (jj + 1) * M],
                                     start=(j == 0), stop=(j == KT - 1))
                if b == 1:
                    # copy out of PSUM, pair bias folded in (frees the banks)
                    nc.vector.tensor_add(pxs[p][:, 0:512], pt0[:],
                                         bst[:, p * M: p * M + 512])
                    nc.vector.tensor_add(pxs[p][:, 512:M], pt1[:],
                                         bst[:, p * M + 512: (p + 1) * M])

            def pair_h(p, actname, pad=(), fine_last=False):
                pt0 = psp.tile([P, 512], FP, tag="bank", name=f"p{p}hb0")
                pt1 = psp.tile([P, 512], FP, tag="bank", name=f"p{p}hb1")
                # fine_last: last pair's final chunk split in two so its
                # matmuls start before the whole MB lands
                chunks = [(0, 4), (4, 4)] if not fine_last else \
                    [(0, 4), (4, 2), (6, 2)]
                for ci, (j0, nj) in enumerate(chunks):
                    if ci == 1:
                        for j_pad in pad:  # late PE padding vs DMA jitter
                            mm1h_col(j_pad)
                    rt = rp.tile([P, nj * M], BF, tag="r" if nj == 4 else "rs",
                                 name=f"p{p}h_r{ci}")
                    nc.sync.dma_start(rt[:], rh_d[p][:, j0 * M:(j0 + nj) * M])
                    for jj in range(nj):
                        j = j0 + jj
                        lhsT = ats["h"][j][:, ts(p, P)]
                        nc.tensor.matmul(pt0[:], lhsT,
                                         rt[:, jj * M: jj * M + 512],
                                         start=(j == 0), stop=(j == KT - 1))
                        nc.tensor.matmul(pt1[:], lhsT,
                                         rt[:, jj * M + 512: (jj + 1) * M],
                                         start=(j == 0), stop=(j == KT - 1))
                gt = gates[p]
                af = getattr(AF, actname)
                quarters = 2 if p == 3 else 1  # fine-grain the o-gate tail
                for bb in range(2):
                    pt = (pt0, pt1)[bb]
                    for q in range(quarters):
                        w = 512 // quarters
                        lo = bb * 512 + q * w
                        nc.vector.tensor_add(pt[:, ts(q, w)], pt[:, ts(q, w)],
                                             pxs[p][:, lo: lo + w])
                        nc.scalar.activation(gt[:, lo: lo + w],
                                             pt[:, ts(q, w)], af)

            # Four mm1-h columns up front bridge the PE over the front-stream
            # tail (first R chunk lands ~6us after mm1-x ends); the rest are
            # interleaved with the first x-chunks so the PE tracks just
            # behind the R arrival front with no HAM-tripping idle gaps.
            for j in range(4):
                mm1h_col(j)
            pair_x_chunk(0, 0)
            pair_x_chunk(0, 1)
            mm1h_col(4)
            pair_x_chunk(1, 0)
            mm1h_col(5)
            pair_x_chunk(1, 1)
            pair_x_chunk(2, 0)
            pair_x_chunk(2, 1)
            pair_x_chunk(3, 0)
            pair_x_chunk(3, 1)

            pair_h(0, PAIRS[0][2], pad=(6, 7))
            pair_h(1, PAIRS[1][2])
            pair_h(2, PAIRS[2][2])
            gi, gf, gg = gates[0], gates[1], gates[2]

            # c_new chain overlaps the o-gate matmuls; stores go on the
            # Activation HWDGE ring to keep the SP ring free for R streaming
            fc = ew.tile([P, M], FP, tag="fc")
            ig = ew.tile([P, M], FP, tag="ig")
            cn = ew.tile([P, M], BF, tag="cn")
            tch_t = ew.tile([P, M], BF, tag="tch")
            for hf in range(2):
                sl = ts(hf, 512)
                nc.vector.tensor_mul(fc[:, sl], gf[:, sl], cs[:, sl])
                nc.vector.tensor_mul(ig[:, sl], gi[:, sl], gg[:, sl])
                nc.vector.tensor_add(cn[:, sl], fc[:, sl], ig[:, sl])
                nc.scalar.dma_start(cn_d[:, sl], cn[:, sl])
                nc.scalar.activation(tch_t[:, sl], cn[:, sl], AF.Tanh)

            pair_h(3, PAIRS[3][2], fine_last=True)  # o
            # dummy matmuls keep the PE "busy" through the elementwise tail so
            # the HAM power manager doesn't throttle the chain + end barrier
            wpd = psp.tile([P, 512], FP, tag="bank", name="tailpad_ps")
            for w in range(14):
                nc.tensor.matmul(wpd[:], wa[:], wb[:], start=True, stop=True,
                                 skip_group_check=True)
            go = gates[3]
            hn = ew.tile([P, M], BF, tag="hn")
            for qf in range(4):  # quarters: shortens the post-last-matmul tail
                sl = ts(qf, 256)
                nc.vector.tensor_mul(hn[:, sl], go[:, sl], tch_t[:, sl])
                nc.scalar.dma_start(hn_d[:, sl], hn[:, sl])

    nc.compile()
    return nc


def _get_program():
    if "nc" not in _cache:
        _cache["nc"] = _build_program()
    return _cache["nc"]


def _pack(a):
    # [R*128, C] -> [128, R*C]; out[p, k*C+c] = a[k*128+p, c]
    r = a.shape[0] // P
    return np.ascontiguousarray(
        a.reshape(r, P, a.shape[1]).transpose(1, 0, 2).reshape(P, r * a.shape[1]))


def _prep_in_maps(inputs):
    import ml_dtypes
    BF = ml_dtypes.bfloat16
    bf = lambda a: np.asarray(a, dtype=np.float32).astype(BF)
    xp = _pack(bf(inputs["x"]))
    hp = _pack(bf(inputs["h"]))
    c = np.asarray(inputs["c"], dtype=np.float32)
    LTx = [bf(np.asarray(inputs[f"L_{xg}"]).T) for xg, _, _ in PAIRS]
    LTh = [bf(np.asarray(inputs[f"L_{hg}"]).T) for _, hg, _ in PAIRS]
    Rxp = [_pack(bf(inputs[f"R_{xg}"])) for xg, _, _ in PAIRS]
    Rhp = [_pack(bf(inputs[f"R_{hg}"])) for _, hg, _ in PAIRS]
    bsum = [(np.asarray(inputs[f"b_{xg}"], dtype=np.float32)
             + np.asarray(inputs[f"b_{hg}"], dtype=np.float32)).astype(BF)
            for xg, hg, _ in PAIRS]

    in_maps = []
    for k in range(NC):
        sl = slice(P * k, P * (k + 1))
        im = {
            "xp": xp, "hp": hp,
            "ltxp": _pack(np.ascontiguousarray(
                np.concatenate([lt[:, sl] for lt in LTx], axis=1))),
            "lthp": _pack(np.ascontiguousarray(
                np.concatenate([lt[:, sl] for lt in LTh], axis=1))),
            "bsp": _pack(np.ascontiguousarray(
                np.concatenate([b[sl] for b in bsum], axis=0))),
            "cprev": np.ascontiguousarray(c[sl].astype(BF)),
        }
        for p in range(4):
            im[f"rxp{p}"] = Rxp[p]
            im[f"rhp{p}"] = Rhp[p]
        in_maps.append(im)
    return in_maps


def kernel(**inputs):
    from concourse.bass_utils import run_bass_kernel_spmd

    nc = _get_program()
    in_maps = _prep_in_maps(inputs)
    res = run_bass_kernel_spmd(nc, in_maps, core_ids=list(range(NC)))
    h_new = np.concatenate(
        [np.asarray(res.results[k]["h_new"], dtype=np.float32) for k in range(NC)],
        axis=0)
    c_new = np.concatenate(
        [np.asarray(res.results[k]["c_new"], dtype=np.float32) for k in range(NC)],
        axis=0)
    return (h_new, c_new)


# revision 24
# speedup vs baseline: 1.1034x; 1.1034x over previous
"""KroneckerLSTM trn2 kernel.

Computes, for 8 gate-klins (L @ t @ R + b, t in {x,h}):
    i = sigmoid(klin_ii(x) + klin_hi(h)); f = sigmoid(...); g = tanh(...); o = sigmoid(...)
    c_new = f*c + i*g ; h_new = o*tanh(c_new)
Returns (h_new, c_new), each [1024,1024] f32.

Sharding: output rows split across 8 cores (128 rows each) -> zero collectives.
Per core, for each gate g:  B_g[rows,:] = (L_g[rows,:] @ t) @ R_g
  mm1 computes A^T directly (lhsT = t tiles (natural), rhs = host-pretransposed
  L^T column-slices, stacked 4 gates wide so N=512), so mm1's PSUM output is the
  lhsT for mm2 (rhs = R_g in natural layout).  The x-klin and h-klin of each
  gate pair accumulate into the same PSUM bank; bias is added in-place in PSUM.

All DMA'd tensors are bf16 (host-side cast): per-core HBM traffic is ~24MB.
DMA-instruction issue is descriptor-rate-limited (~5ns per partition line), so
all big tensors are host-packed into [128, K*freedim] layouts whose partition
lines are 8KB-contiguous in DRAM; each DMA instruction then moves 0.5-2MB with
only 128 descriptor lines.  Input loads stream on the SP HWDGE ring in
consumption order; output stores use the Activation ring so they never block
input streaming.
"""

import sys

import numpy as np

if "/opt/trn_rl_repo" not in sys.path:
    sys.path.insert(0, "/opt/trn_rl_repo")

N = 1024
M = 1024
P = 128
NC = 8
KT = N // P  # 8 k-tiles of 128
# gate pairs in order i, f, g, o: (x-gate, h-gate, activation)
PAIRS = [("ii", "hi", "Sigmoid"), ("if", "hf", "Sigmoid"),
         ("ig", "hg", "Tanh"), ("io", "ho", "Sigmoid")]

_cache = {}


def _build_program():
    import concourse.bass as bass
    import concourse.mybir as mybir
    import concourse.tile as tile
    from concourse import bacc
    from concourse.bass import ts

    FP = mybir.dt.float32
    BF = mybir.dt.bfloat16
    AF = mybir.ActivationFunctionType

    nc = bacc.Bacc("TRN2", target_bir_lowering=False, debug=False,
                   enable_asserts=False, num_devices=NC)

    # packed layouts: [128, K*freedim], partition p / chunk k = source row k*128+p
    xp_d = nc.dram_tensor("xp", [P, KT * M], BF, kind="ExternalInput").ap()
    hp_d = nc.dram_tensor("hp", [P, KT * M], BF, kind="ExternalInput").ap()
    ltx_d = nc.dram_tensor("ltxp", [P, KT * 512], BF, kind="ExternalInput").ap()
    lth_d = nc.dram_tensor("lthp", [P, KT * 512], BF, kind="ExternalInput").ap()
    rx_d = [nc.dram_tensor(f"rxp{p}", [P, KT * M], BF, kind="ExternalInput").ap()
            for p in range(4)]
    rh_d = [nc.dram_tensor(f"rhp{p}", [P, KT * M], BF, kind="ExternalInput").ap()
            for p in range(4)]
    bs_d = nc.dram_tensor("bsp", [P, 4 * M], BF, kind="ExternalInput").ap()
    c_d = nc.dram_tensor("cprev", [P, M], BF, kind="ExternalInput").ap()
    hn_d = nc.dram_tensor("h_new", [P, M], BF, kind="ExternalOutput").ap()
    cn_d = nc.dram_tensor("c_new", [P, M], BF, kind="ExternalOutput").ap()

    with tile.TileContext(nc) as tc:
        from contextlib import ExitStack
        with ExitStack() as ctx:
            big = ctx.enter_context(tc.tile_pool(name="big", bufs=1))
            atp = ctx.enter_context(tc.tile_pool(name="at", bufs=1))
            rp = ctx.enter_context(tc.tile_pool(name="rstream", bufs=8))
            psp = ctx.enter_context(tc.tile_pool(name="ps", bufs=8, space="PSUM"))
            gp = ctx.enter_context(tc.tile_pool(name="gates", bufs=1))
            ew = ctx.enter_context(tc.tile_pool(name="ew", bufs=1))
            wp = ctx.enter_context(tc.tile_pool(name="warm", bufs=1))

            # small PE warm-up burst overlapping the DMA prologue
            wa = wp.tile([P, P], BF, tag="wa")
            wb = wp.tile([P, 512], BF, tag="wb")
            nc.vector.memset(wa[:], 0.0)
            nc.vector.memset(wb[:], 0.0)
            wps = psp.tile([P, 512], FP, tag="bank", name="warm_ps")
            for w in range(8):
                nc.tensor.matmul(wps[:], wa[:], wb[:], start=True, stop=True,
                                 skip_group_check=True)

            # ---- all input loads, issued upfront in consumption order (SP
            # ring, FIFO).  One DMA per tile (multi-writer tiles add
            # per-reader semaphore overhead); early tiles are small so mm1
            # starts as soon as possible, later ones big to save issue slots.
            # front loads alternate between the two HWDGE rings (SP + Act) so
            # the early issue rate isn't bound by one ring's 0.67us/instr
            xsplit = [(0, 1), (1, 1), (2, 2), (4, 4)]  # (kc0, n_ktiles)
            ltsplit = [(0, 1), (1, 1), (2, 2), (4, 4)]
            xcs, ltxs = [], []
            for i, ((k0, nk), (l0, nl)) in enumerate(zip(xsplit, ltsplit)):
                lt_t = big.tile([P, nl * 512], BF, tag=f"ltx{i}", name=f"ltx{i}")
                ltxs.append((l0, lt_t))
                nc.scalar.dma_start(lt_t[:], ltx_d[:, l0 * 512:(l0 + nl) * 512])
                t = big.tile([P, nk * M], BF, tag=f"xc{i}", name=f"xc{i}")
                xcs.append((k0, t))
                nc.sync.dma_start(t[:], xp_d[:, k0 * M:(k0 + nk) * M])
            ltht = big.tile([P, KT * 512], BF, tag="lth")
            nc.scalar.dma_start(ltht[:], lth_d[:])
            hcs = []
            for i in range(4):
                t = big.tile([P, 2 * M], BF, tag=f"hc{i}", name=f"hc{i}")
                hcs.append(t)
                (nc.scalar if i % 2 else nc.sync).dma_start(
                    t[:], hp_d[:, 2 * i * M:2 * (i + 1) * M])
            bst = big.tile([P, 4 * M], BF, tag="bs")
            nc.scalar.dma_start(bst[:], bs_d[:])
            cs = ew.tile([P, M], BF, tag="cs")
            nc.scalar.dma_start(cs[:], c_d[:])

            def x_ap(kc, j):  # lhsT slice for x k-tile kc, output column j
                for k0, t in reversed(xcs):
                    if kc >= k0:
                        return t[:, (kc - k0) * M + j * P:
                                 (kc - k0) * M + (j + 1) * P]

            def ltx_ap(kc):
                for l0, t in reversed(ltxs):
                    if kc >= l0:
                        return t[:, ts(kc - l0, 512)]

            # mm1-x: at_x[j][mloc, 4*128] = sum_k x[k, j*128+mloc] * LTx[k, col]
            ats = {"x": [None] * KT, "h": [None] * KT}
            pts = [psp.tile([P, 4 * P], FP, tag="bank", name=f"pt_x_{j}")
                   for j in range(KT)]
            for kc in range(KT):
                lts = ltx_ap(kc)
                for j in range(KT):
                    nc.tensor.matmul(pts[j][:], x_ap(kc, j), lts,
                                     start=(kc == 0), stop=(kc == KT - 1))
            for j in range(KT):
                at = atp.tile([P, 4 * P], BF, tag=f"atx{j}", name=f"atx{j}")
                nc.vector.tensor_copy(at[:], pts[j][:])
                ats["x"][j] = at

            # SBUF fp32 partials for the mm2 x-passes (bias folded in)
            pxp = ctx.enter_context(tc.tile_pool(name="px", bufs=1))
            pxs = [pxp.tile([P, M], BF, tag=f"px{p}", name=f"px{p}")
                   for p in range(4)]
            gates = [gp.tile([P, M], BF, tag=f"g{p}", name=f"g{p}")
                     for p in range(4)]

            def mm1h_col(j):
                # one mm1-h output column j (8 MMs, depends only on h/lth):
                # padding work in front of each R-chunk wait so the PE never
                # idles long enough to trip the HAM power throttle
                pw = psp.tile([P, 4 * P], FP, tag="bank", name=f"pt_h_{j}")
                for kc in range(KT):
                    nc.tensor.matmul(pw[:], hcs[kc // 2][:, (kc % 2) * M + j * P:
                                                         (kc % 2) * M + (j + 1) * P],
                                     ltht[:, ts(kc, 512)],
                                     start=(kc == 0), stop=(kc == KT - 1))
                at = atp.tile([P, 4 * P], BF, tag=f"ath{j}", name=f"ath{j}")
                nc.vector.tensor_copy(at[:], pw[:])
                ats["h"][j] = at

            px_banks = {}

            def pair_x_chunk(p, b):
                if b == 0:
                    px_banks[p] = (
                        psp.tile([P, 512], FP, tag="bank", name=f"p{p}xb0"),
                        psp.tile([P, 512], FP, tag="bank", name=f"p{p}xb1"))
                pt0, pt1 = px_banks[p]
                rt = rp.tile([P, 4 * M], BF, tag="r")
                nc.sync.dma_start(rt[:], rx_d[p][:, ts(b, 4 * M)])
                for jj in range(4):
                    j = 4 * b + jj
                    lhsT = ats["x"][j][:, ts(p, P)]
                    nc.tensor.matmul(pt0[:], lhsT,
                                     rt[:, jj * M: jj * M + 512],
                                     start=(j == 0), stop=(j == KT - 1))
                    nc.tensor.matmul(pt1[:], lhsT,
                                     rt[:, jj * M + 512: (jj + 1) * M],
                                     start=(j == 0), stop=(j == KT - 1))
                if b == 1:
                    # copy out of PSUM, pair bias folded in (frees the banks)
                    nc.vector.tensor_add(pxs[p][:, 0:512], pt0[:],
                                         bst[:, p * M: p * M + 512])
                    nc.vector.tensor_add(pxs[p][:, 512:M], pt1[:],
                                         bst[:, p * M + 512: (p + 1) * M])

            def pair_h(p, actname, pad=(), fine_last=False):
                pt0 = psp.tile([P, 512], FP, tag="bank", name=f"p{p}hb0")
                pt1 = psp.tile([P, 512], FP, tag="bank", name=f"p{p}hb1")
                # fine_last: last pair's final chunk split in two so its
                # matmuls start before the whole MB lands
                chunks = [(0, 4), (4, 4)] if not fine_last else \
                    [(0, 4), (4, 2), (6, 2)]
                for ci, (j0, nj) in enumerate(chunks):
                    if ci == 1:
                        for j_pad in pad:  # late PE padding vs DMA jitter
                            mm1h_col(j_pad)
                    rt = rp.tile([P, nj * M], BF, tag="r" if nj == 4 else "rs",
                                 name=f"p{p}h_r{ci}")
                    nc.sync.dma_start(rt[:], rh_d[p][:, j0 * M:(j0 + nj) * M])
                    for jj in range(nj):
                        j = j0 + jj
                        lhsT = ats["h"][j][:, ts(p, P)]
                        nc.tensor.matmul(pt0[:], lhsT,
                                         rt[:, jj * M: jj * M + 512],
                                         start=(j == 0), stop=(j == KT - 1))
                        nc.tensor.matmul(pt1[:], lhsT,
                                         rt[:, jj * M + 512: (jj + 1) * M],
                                         start=(j == 0), stop=(j == KT - 1))
                gt = gates[p]
                af = getattr(AF, actname)
                quarters = 2 if p == 3 else 1  # fine-grain the o-gate tail
                for bb in range(2):
                    pt = (pt0, pt1)[bb]
                    for q in range(quarters):
                        w = 512 // quarters
                        lo = bb * 512 + q * w
                        nc.vector.tensor_add(pt[:, ts(q, w)], pt[:, ts(q, w)],
                                             pxs[p][:, lo: lo + w])
                        nc.scalar.activation(gt[:, lo: lo + w],
                                             pt[:, ts(q, w)], af)

            # Four mm1-h columns up front bridge the PE over the front-stream
            # tail (first R chunk lands ~6us after mm1-x ends); the rest are
            # interleaved with the first x-chunks so the PE tracks just
            # behind the R arrival front with no HAM-tripping idle gaps.
            for j in range(4):
                mm1h_col(j)
            pair_x_chunk(0, 0)
            pair_x_chunk(0, 1)
            mm1h_col(4)
            pair_x_chunk(1, 0)
            mm1h_col(5)
            pair_x_chunk(1, 1)
            pair_x_chunk(2, 0)
            pair_x_chunk(2, 1)
            pair_x_chunk(3, 0)
            pair_x_chunk(3, 1)

            pair_h(0, PAIRS[0][2], pad=(6, 7))
            pair_h(1, PAIRS[1][2])
            pair_h(2, PAIRS[2][2])
            gi, gf, gg = gates[0], gates[1], gates[2]

            # c_new chain overlaps the o-gate matmuls; stores go on the
            # Activation HWDGE ring to keep the SP ring free for R streaming
            fc = ew.tile([P, M], FP, tag="fc")
            ig = ew.tile([P, M], FP, tag="ig")
            cn = ew.tile([P, M], BF, tag="cn")
            tch_t = ew.tile([P, M], BF, tag="tch")
            for hf in range(2):
                sl = ts(hf, 512)
                nc.vector.tensor_mul(fc[:, sl], gf[:, sl], cs[:, sl])
                nc.vector.tensor_mul(ig[:, sl], gi[:, sl], gg[:, sl])
                nc.vector.tensor_add(cn[:, sl], fc[:, sl], ig[:, sl])
                nc.scalar.dma_start(cn_d[:, sl], cn[:, sl])
                nc.scalar.activation(tch_t[:, sl], cn[:, sl], AF.Tanh)

            pair_h(3, PAIRS[3][2], fine_last=True)  # o
            # dummy matmuls keep the PE "busy" through the elementwise tail so
            # the HAM power manager doesn't throttle the chain + end barrier
            wpd = psp.tile([P, 512], FP, tag="bank", name="tailpad_ps")
            for w in range(14):
                nc.tensor.matmul(wpd[:], wa[:], wb[:], start=True, stop=True,
                                 skip_group_check=True)
            go = gates[3]
            hn = ew.tile([P, M], BF, tag="hn")
            for qf in range(4):  # quarters: shortens the post-last-matmul tail
                sl = ts(qf, 256)
                nc.vector.tensor_mul(hn[:, sl], go[:, sl], tch_t[:, sl])
                nc.scalar.dma_start(hn_d[:, sl], hn[:, sl])

    nc.compile()
    return nc


def _get_program():
    if "nc" not in _cache:
        _cache["nc"] = _build_program()
    return _cache["nc"]


def _pack(a):
    # [R*128, C] -> [128, R*C]; out[p, k*C+c] = a[k*128+p, c]
    r = a.shape[0] // P
    return np.ascontiguousarray(
        a.reshape(r, P, a.shape[1]).transpose(1, 0, 2).reshape(P, r * a.shape[1]))


def _prep_in_maps(inputs):
    import ml_dtypes
    BF = ml_dtypes.bfloat16
    bf = lambda a: np.asarray(a, dtype=np.float32).astype(BF)
    xp = _pack(bf(inputs["x"]))
    hp = _pack(bf(inputs["h"]))
    c = np.asarray(inputs["c"], dtype=np.float32)
    LTx = [bf(np.asarray(inputs[f"L_{xg}"]).T) for xg, _, _ in PAIRS]
    LTh = [bf(np.asarray(inputs[f"L_{hg}"]).T) for _, hg, _ in PAIRS]
    Rxp = [_pack(bf(inputs[f"R_{xg}"])) for xg, _, _ in PAIRS]
    Rhp = [_pack(bf(inputs[f"R_{hg}"])) for _, hg, _ in PAIRS]
    bsum = [(np.asarray(inputs[f"b_{xg}"], dtype=np.float32)
             + np.asarray(inputs[f"b_{hg}"], dtype=np.float32)).astype(BF)
            for xg, hg, _ in PAIRS]

    in_maps = []
    for k in range(NC):
        sl = slice(P * k, P * (k + 1))
        im = {
            "xp": xp, "hp": hp,
            "ltxp": _pack(np.ascontiguousarray(
                np.concatenate([lt[:, sl] for lt in LTx], axis=1))),
            "lthp": _pack(np.ascontiguousarray(
                np.concatenate([lt[:, sl] for lt in LTh], axis=1))),
            "bsp": _pack(np.ascontiguousarray(
                np.concatenate([b[sl] for b in bsum], axis=0))),
            "cprev": np.ascontiguousarray(c[sl].astype(BF)),
        }
        for p in range(4):
            im[f"rxp{p}"] = Rxp[p]
            im[f"rhp{p}"] = Rhp[p]
        in_maps.append(im)
    return in_maps


def kernel(**inputs):
    from concourse.bass_utils import run_bass_kernel_spmd

    nc = _get_program()
    in_maps = _prep_in_maps(inputs)
    res = run_bass_kernel_spmd(nc, in_maps, core_ids=list(range(NC)))
    h_new = np.concatenate(
        [np.asarray(res.results[k]["h_new"], dtype=np.float32) for k in range(NC)],
        axis=0)
    c_new = np.concatenate(
        [np.asarray(res.results[k]["c_new"], dtype=np.float32) for k in range(NC)],
        axis=0)
    return (h_new, c_new)
